# revision 1
# baseline (speedup 1.0000x reference)
"""Trainium2 Bass kernel for nn_Compressor (sparse_attention compressor).

Computes, for x [B=2, S=8192, D=4096]:
  kv = x @ wkv, sc = x @ wgate               (HEAD_DIM=512)
  w  = softmax(sc + ape, axis=window-of-128) (windows of RATIO=128 tokens)
  comp = sum(w * kv, axis=window)            -> [B, 64, 512]
  RMSNorm(comp) * norm_weight, RoPE on last 64 dims at window-start pos
  scatter into paged cache [4, 64, 512] via block_offsets

Sharding: 128 windows (B*NC) split across 8 cores, 16 windows each.
Pure data parallel - weights replicated, no collectives.
"""

import sys

sys.path.insert(0, "/opt/trn_rl_repo")

import numpy as np
from contextlib import ExitStack

import concourse.bass as bass
import concourse.tile as tile
from concourse import bacc, mybir
from concourse.masks import make_identity

# Problem constants (hardcoded per contract - kernel.py must be self-contained)
B = 2
S = 8192
D = 4096
HEAD = 512
RATIO = 128
RD = 64
EPS = 1e-6
NC_PER_SEQ = S // RATIO  # 64
BLOCK_SIZE = 8192
ENTRIES = 64
NUM_BLOCKS = 4
N_CORES = 8
N_WIN_TOTAL = B * NC_PER_SEQ  # 128
N_WIN = N_WIN_TOTAL // N_CORES  # 16 windows per core

F32 = mybir.dt.float32
F32R = mybir.dt.float32r


def build_program(
    n_win=N_WIN,
    d=D,
    head=HEAD,
    rd=RD,
    mm_f32r=True,
    tr_f32r=True,
    debug=False,
):
    """Build the per-core Bass program (SPMD: identical on all cores)."""
    assert d % 128 == 0 and head == 512
    DC = d // 128  # number of 128-wide contraction chunks
    # transpose-batch: how many 128x128 transposes share one PSUM bank
    TB = 4 if DC % 4 == 0 else (2 if DC % 2 == 0 else 1)
    NB = DC // TB  # transpose batches per window
    # x loaded in halves to bound SBUF (each [128, d//xsplit])
    XSPLIT = 2 if d >= 4096 else 1
    XW = d // XSPLIT

    nc = bacc.Bacc(None, target_bir_lowering=False, debug=debug)

    xdt = F32R if (mm_f32r and tr_f32r) else F32
    x_h = nc.declare_dram_parameter("x", [n_win * RATIO, d], xdt, isOutput=False)
    wdt_p = F32R if mm_f32r else F32
    wkv_h = nc.declare_dram_parameter("wkv", [d, head], wdt_p, isOutput=False)
    wg_h = nc.declare_dram_parameter("wgate", [d, head], wdt_p, isOutput=False)
    ape_h = nc.declare_dram_parameter("ape", [RATIO, head], F32, isOutput=False)
    nw_h = nc.declare_dram_parameter("nw", [head], F32, isOutput=False)
    cos_h = nc.declare_dram_parameter("cosr", [n_win, rd // 2], F32, isOutput=False)
    sin_h = nc.declare_dram_parameter("sinr", [n_win, rd // 2], F32, isOutput=False)
    out_h = nc.declare_dram_parameter("out", [n_win, head], F32, isOutput=True)

    # weight chunk-group size (d-chunks per DMA) - chunked so the first
    # projection matmuls only wait on the first small DMA
    WG = 4 if DC % 4 == 0 else 1
    NWG = DC // WG

    with TileProgram(nc) as tc:
        with ExitStack() as ctx:
            consts = ctx.enter_context(tc.tile_pool(name="consts", bufs=1))
            wpool = ctx.enter_context(tc.tile_pool(name="wpool", bufs=1))
            xin_p = ctx.enter_context(tc.tile_pool(name="xin", bufs=3))
            xtp_p = ctx.enter_context(tc.tile_pool(name="xtp", bufs=6))
            eb_p = ctx.enter_context(tc.tile_pool(name="eb", bufs=2))
            epi_p = ctx.enter_context(tc.tile_pool(name="epi", bufs=4))
            sm_p = ctx.enter_context(tc.tile_pool(name="sm", bufs=2))
            ps_t = ctx.enter_context(tc.tile_pool(name="ps_t", bufs=2, space="PSUM"))
            ps_mm = ctx.enter_context(tc.tile_pool(name="ps_mm", bufs=2, space="PSUM"))
            ps_v = ctx.enter_context(tc.tile_pool(name="ps_v", bufs=1, space="PSUM"))

            # ---- constants ----
            ident0 = consts.tile([128, 128], F32)
            make_identity(nc, ident0)
            if tr_f32r:
                ident = consts.tile([128, 128], F32R)
                nc.vector.tensor_copy(ident, ident0)
            else:
                ident = ident0

            # striped indicator: big[:, (n_win+1)*w] column of ones; slice
            # big[:, n_win*w : n_win*w+n_win] is E_w with ones in column w.
            big0 = consts.tile([128, n_win * n_win], F32)
            nc.vector.memset(big0, 0.0)
            for w in range(n_win):
                nc.vector.memset(big0[:, (n_win + 1) * w : (n_win + 1) * w + 1], 1.0)
            big = consts.tile([128, n_win * n_win], F32R if mm_f32r else F32)
            nc.vector.tensor_copy(big, big0)

            ape_sb = consts.tile([RATIO, head], F32)
            nc.sync.dma_start(out=ape_sb, in_=ape_h[:, :])

            # norm_weight broadcast across the n_win partitions
            nw_sb = consts.tile([n_win, head], F32)
            nw_ap = nw_h[:]
            nw_bcast = bass.AP(
                tensor=nw_ap.tensor, offset=nw_ap.offset,
                ap=[[0, n_win]] + list(nw_ap.ap),
            )
            nc.sync.dma_start(out=nw_sb, in_=nw_bcast)

            cos_sb = consts.tile([n_win, rd // 2], F32)
            nc.sync.dma_start(out=cos_sb, in_=cos_h[:, :])
            sin_sb = consts.tile([n_win, rd // 2], F32)
            nc.sync.dma_start(out=sin_sb, in_=sin_h[:, :])

            # ---- weights first (chunked cast-DMAs, earliest d-chunks first,
            # separate tiles so each chunk's consumers unblock independently)
            wdt = F32R if mm_f32r else F32
            wdma = nc.gpsimd
            wkv_view = wkv_h[:, :].rearrange("(c p) h -> p c h", p=128)
            wg_view = wg_h[:, :].rearrange("(c p) h -> p c h", p=128)
            wkv_g = []
            wg_g = []
            for g in range(NWG):
                t = wpool.tile([128, WG, head], wdt, tag=f"wkv{g}")
                wdma.dma_start(out=t, in_=wkv_view[:, g * WG : (g + 1) * WG, :])
                wkv_g.append(t)
                t = wpool.tile([128, WG, head], wdt, tag=f"wg{g}")
                wdma.dma_start(out=t, in_=wg_view[:, g * WG : (g + 1) * WG, :])
                wg_g.append(t)

            def wkv_chunk(dc):
                return wkv_g[dc // WG][:, dc % WG, :]

            def wg_chunk(dc):
                return wg_g[dc // WG][:, dc % WG, :]

            # persistent PSUM collectors for pooled numerator / denominator
            den_ps = ps_v.tile([n_win, head], F32, tag="den")
            num_ps = ps_v.tile([n_win, head], F32, tag="num")

            x_view = x_h[:, :].rearrange("(w p) d -> w p d", p=RATIO)

            for w in range(n_win):
                # load x window [128 tokens, d] in XSPLIT pieces
                xh = []
                for s in range(XSPLIT):
                    t = xin_p.tile([RATIO, XW], F32R if tr_f32r else F32, tag="xin")
                    nc.sync.dma_start(
                        out=t, in_=x_view[w, :, s * XW : (s + 1) * XW]
                    )
                    xh.append(t)

                kv_ps = ps_mm.tile([RATIO, head], F32, tag="kv")
                sc_ps = ps_mm.tile([RATIO, head], F32, tag="sc")

                chunks_per_half = DC // XSPLIT
                for b in range(NB):
                    tps = ps_t.tile([128, TB * 128], F32R if tr_f32r else F32, tag="tps")
                    for k in range(TB):
                        dc = b * TB + k
                        src = xh[dc // chunks_per_half]
                        off = (dc % chunks_per_half) * 128
                        nc.tensor.matmul(
                            tps[:, k * 128 : (k + 1) * 128],
                            src[:, off : off + 128],
                            ident,
                            is_transpose=True,
                        )
                    xts = xtp_p.tile([128, TB * 128], F32R if mm_f32r else F32, tag="xts")
                    nc.any.tensor_copy(xts, tps)
                    for k in range(TB):
                        dc = b * TB + k
                        lhsT = xts[:, k * 128 : (k + 1) * 128]
                        nc.tensor.matmul(
                            kv_ps, lhsT, wkv_chunk(dc),
                            start=(dc == 0), stop=(dc == DC - 1),
                        )
                        nc.tensor.matmul(
                            sc_ps, lhsT, wg_chunk(dc),
                            start=(dc == 0), stop=(dc == DC - 1),
                        )

                # pooling for this window
                se = eb_p.tile([RATIO, head], F32, tag="se")
                nc.vector.tensor_add(se, sc_ps, ape_sb)
                e = eb_p.tile([RATIO, head], F32R if mm_f32r else F32, tag="e")
                nc.scalar.activation(e, se, mybir.ActivationFunctionType.Exp)
                m = eb_p.tile([RATIO, head], F32R if mm_f32r else F32, tag="m")
                nc.vector.tensor_mul(m, e, kv_ps)

                ind = big[:, n_win * w : n_win * w + n_win]
                nc.tensor.matmul(
                    den_ps, ind, e,
                    start=(w == 0), stop=(w == n_win - 1),
                )
                nc.tensor.matmul(
                    num_ps, ind, m,
                    start=(w == 0), stop=(w == n_win - 1),
                )

            # ---- epilogue (batched over all n_win windows) ----
            dens = epi_p.tile([n_win, head], F32, tag="ep")
            nc.any.tensor_copy(dens, den_ps)
            rden = epi_p.tile([n_win, head], F32, tag="ep")
            nc.vector.reciprocal(rden, dens)
            comp = epi_p.tile([n_win, head], F32, tag="ep")
            nc.vector.tensor_mul(comp, num_ps, rden)

            # RMSNorm: rstd = 1/sqrt(mean(comp^2) + eps)
            sq = epi_p.tile([n_win, head], F32, tag="ep")
            ssq = sm_p.tile([n_win, 1], F32, tag="ssq")
            nc.scalar.activation(
                sq, comp, mybir.ActivationFunctionType.Square, accum_out=ssq
            )
            eps_sb = sm_p.tile([n_win, 1], F32, tag="eps")
            nc.vector.memset(eps_sb, EPS)
            srt = sm_p.tile([n_win, 1], F32, tag="srt")
            nc.scalar.activation(
                srt, ssq, mybir.ActivationFunctionType.Sqrt,
                scale=1.0 / head, bias=eps_sb,
            )
            rstd = sm_p.tile([n_win, 1], F32, tag="rstd")
            nc.vector.reciprocal(rstd, srt)

            compn = epi_p.tile([n_win, head], F32, tag="ep")
            nc.scalar.activation(
                compn, comp, mybir.ActivationFunctionType.Copy, scale=rstd
            )
            final = epi_p.tile([n_win, head], F32, tag="ep")
            nc.vector.tensor_mul(final, compn, nw_sb)

            # RoPE on last rd dims, pairs interleaved along free dim
            pr = final[:, head - rd : head].rearrange("p (k two) -> p k two", two=2)
            ev = pr[:, :, 0]
            od = pr[:, :, 1]
            t1 = sm_p.tile([n_win, rd // 2], F32, tag="t1")
            t2 = sm_p.tile([n_win, rd // 2], F32, tag="t2")
            t3 = sm_p.tile([n_win, rd // 2], F32, tag="t3")
            t4 = sm_p.tile([n_win, rd // 2], F32, tag="t4")
            nc.vector.tensor_mul(t1, ev, cos_sb)
            nc.vector.tensor_mul(t2, od, sin_sb)
            nc.vector.tensor_mul(t3, ev, sin_sb)
            nc.vector.tensor_mul(t4, od, cos_sb)
            nc.vector.tensor_sub(ev, t1, t2)
            nc.vector.tensor_add(od, t3, t4)

            nc.sync.dma_start(out=out_h[:, :], in_=final)

    nc.compile()
    return nc


def TileProgram(nc):
    return tile.TileContext(nc)


_cache = {}


def _get_program():
    if "nc" not in _cache:
        _cache["nc"] = build_program()
    return _cache["nc"]


def make_input_maps(x, wkv, wgate, ape, norm_weight, cos, sin):
    xf = np.ascontiguousarray(x.reshape(B * S, D))
    tok_per_core = N_WIN * RATIO
    maps = []
    for i in range(N_CORES):
        g0 = i * N_WIN
        # position of window start within its sequence
        pos = ((np.arange(g0, g0 + N_WIN) % NC_PER_SEQ) * RATIO).astype(np.int64)
        maps.append(
            {
                "x": np.ascontiguousarray(
                    xf[i * tok_per_core : (i + 1) * tok_per_core]
                ),
                "wkv": np.ascontiguousarray(wkv),
                "wgate": np.ascontiguousarray(wgate),
                "ape": np.ascontiguousarray(ape),
                "nw": np.ascontiguousarray(norm_weight),
                "cosr": np.ascontiguousarray(cos[pos]),
                "sinr": np.ascontiguousarray(sin[pos]),
            }
        )
    return maps


def assemble_output(results, block_offsets):
    cache = np.zeros((NUM_BLOCKS, ENTRIES, HEAD), np.float32)
    bo = np.asarray(block_offsets)
    for i in range(N_CORES):
        comp = results[i]["out"]  # [N_WIN, HEAD]
        g0 = i * N_WIN
        b = g0 // NC_PER_SEQ
        c0 = g0 % NC_PER_SEQ
        cache[int(bo[b, 0]), c0 : c0 + N_WIN, :] = comp
    return cache


def _ensure_ntff_hook():
    """Register the axon NTFF profile hook if the image's antenv lacks it."""
    import types

    try:
        from antenv.axon_hooks import get_axon_ntff_profile_hook  # noqa: F401

        return
    except ImportError:
        pass
    import antenv
    from trn_agent_boot.trn_boot import _ntff_profile_via_ctypes

    hook = _ntff_profile_via_ctypes("/opt/axon/libaxon_pjrt.so")
    mod = types.ModuleType("antenv.axon_hooks")
    state = {"hook": hook}
    mod.get_axon_ntff_profile_hook = lambda: state["hook"]
    mod.set_axon_ntff_profile_hook = lambda h: state.__setitem__("hook", h)
    sys.modules["antenv.axon_hooks"] = mod
    antenv.axon_hooks = mod


def _patch_ldw_opt():
    """Flip walrus --enable-ldw-opt to true (env BASS_LDW_OPT=1)."""
    import os

    if os.environ.get("BASS_LDW_OPT") == "0":
        return
    from concourse import bass_utils as bu

    if getattr(bu, "_ldw_patched", False):
        return
    orig = bu.run_command

    def patched(cmd, **kw):
        cmd = [
            c.replace("--enable-ldw-opt=false", "--enable-ldw-opt=true")
            if isinstance(c, str) else c
            for c in cmd
        ]
        return orig(cmd, **kw)

    bu.run_command = patched
    bu._ldw_patched = True


def run_full(inputs, trace=False):
    from concourse.bass_utils import run_bass_kernel_spmd

    _patch_ldw_opt()
    if trace:
        _ensure_ntff_hook()
    nc = _get_program()
    maps = make_input_maps(
        inputs["x"], inputs["wkv"], inputs["wgate"], inputs["ape"],
        inputs["norm_weight"], inputs["cos"], inputs["sin"],
    )
    r = run_bass_kernel_spmd(nc, maps, list(range(N_CORES)), trace=trace)
    out = assemble_output(r.results, inputs["block_offsets"])
    return out, r


def kernel(**inputs):
    inputs = {k: np.asarray(v) for k, v in inputs.items()}
    out, _ = run_full(inputs, trace=False)
    return out



# revision 9
# speedup vs baseline: 1.5961x; 1.5961x over previous
"""Trainium2 Bass kernel for nn_Compressor (sparse_attention compressor).

Computes, for x [B=2, S=8192, D=4096]:
  kv = x @ wkv, sc = x @ wgate               (HEAD_DIM=512)
  w  = softmax(sc + ape, axis=window-of-128) (windows of RATIO=128 tokens)
  comp = sum(w * kv, axis=window)            -> [B, 64, 512]
  RMSNorm(comp) * norm_weight, RoPE on last 64 dims at window-start pos
  scatter into paged cache [4, 64, 512] via block_offsets

Sharding: 128 windows (B*NC) split across 8 cores, 16 windows each.
Pure data parallel - weights replicated, no collectives.

x is pre-transposed on the host to [D, tokens] per core, so the device
does no transposes: each 128-d chunk of xT is the matmul stationary
directly. All matmul operands are bf16 (host-cast). The ape bias is
folded into the sc accumulation as an extra identity-stationary matmul.
Per-window softmax pooling runs one window behind the projections so
the PE never waits on the exp/mul chain.
"""

import sys

sys.path.insert(0, "/opt/trn_rl_repo")

import numpy as np
import ml_dtypes
from contextlib import ExitStack

import concourse.bass as bass
import concourse.tile as tile
from concourse import bacc, mybir
from concourse.masks import make_identity

# Problem constants (hardcoded per contract - kernel.py must be self-contained)
B = 2
S = 8192
D = 4096
HEAD = 512
RATIO = 128
RD = 64
EPS = 1e-6
NC_PER_SEQ = S // RATIO  # 64
BLOCK_SIZE = 8192
ENTRIES = 64
NUM_BLOCKS = 4
N_CORES = 8
N_WIN_TOTAL = B * NC_PER_SEQ  # 128
N_WIN = N_WIN_TOTAL // N_CORES  # 16 windows per core
TOK = N_WIN * RATIO  # 2048 tokens per core

F32 = mybir.dt.float32
BF16 = mybir.dt.bfloat16
BF16_NP = ml_dtypes.bfloat16


def build_program(n_win=N_WIN, d=D, head=HEAD, rd=RD, debug=False):
    """Build the per-core Bass program (SPMD: identical on all cores)."""
    assert d % 128 == 0 and head == 512
    DC = d // 128  # 32 contraction chunks
    tok = n_win * RATIO
    NG = 4  # token groups for x DMA granularity
    GT = tok // NG  # 512 tokens per group
    WPG = GT // RATIO  # windows per group

    nc = bacc.Bacc(None, target_bir_lowering=False, debug=debug)

    xT_h = nc.declare_dram_parameter("xT", [d, tok], BF16, isOutput=False)
    # wcat = [wkv | wgate] fused along head dim -> one matmul per chunk
    wcat_h = nc.declare_dram_parameter("wcat", [d, 2 * head], BF16, isOutput=False)
    # apecat = [zeros | ape]: bias fold targets only the sc half
    ape_h = nc.declare_dram_parameter("apecat", [RATIO, 2 * head], BF16, isOutput=False)
    nw_h = nc.declare_dram_parameter("nw", [head], F32, isOutput=False)
    cos_h = nc.declare_dram_parameter("cosr", [n_win, rd // 2], F32, isOutput=False)
    sin_h = nc.declare_dram_parameter("sinr", [n_win, rd // 2], F32, isOutput=False)
    out_h = nc.declare_dram_parameter("out", [n_win, head], F32, isOutput=True)

    with TileProgram(nc) as tc:
        with ExitStack() as ctx:
            consts = ctx.enter_context(tc.tile_pool(name="consts", bufs=1))
            xpool = ctx.enter_context(tc.tile_pool(name="xpool", bufs=3))
            wpool = ctx.enter_context(tc.tile_pool(name="wpool", bufs=1))
            em_p = ctx.enter_context(tc.tile_pool(name="em", bufs=2))
            epi_p = ctx.enter_context(tc.tile_pool(name="epi", bufs=4))
            sm_p = ctx.enter_context(tc.tile_pool(name="sm", bufs=2))
            ps_mm = ctx.enter_context(tc.tile_pool(name="ps_mm", bufs=2, space="PSUM"))
            ps_v = ctx.enter_context(tc.tile_pool(name="ps_v", bufs=1, space="PSUM"))

            # ---- constants ----
            ident0 = consts.tile([128, 128], F32)
            make_identity(nc, ident0)
            ident = consts.tile([128, 128], BF16)
            nc.vector.tensor_copy(ident, ident0)

            # striped indicator: big[:, (n_win+1)*w] column of ones; slice
            # big[:, n_win*w : n_win*w+n_win] is E_w with ones in column w.
            big0 = consts.tile([128, n_win * n_win], F32)
            nc.vector.memset(big0, 0.0)
            for w in range(n_win):
                nc.vector.memset(big0[:, (n_win + 1) * w : (n_win + 1) * w + 1], 1.0)
            big = consts.tile([128, n_win * n_win], BF16)
            nc.vector.tensor_copy(big, big0)

            ape_sb = consts.tile([RATIO, 2 * head], BF16)
            nc.sync.dma_start(out=ape_sb, in_=ape_h[:, :])

            # norm_weight broadcast across the n_win partitions
            nw_sb = consts.tile([n_win, head], F32)
            nw_ap = nw_h[:]
            nw_bcast = bass.AP(
                tensor=nw_ap.tensor, offset=nw_ap.offset,
                ap=[[0, n_win]] + list(nw_ap.ap),
            )
            nc.sync.dma_start(out=nw_sb, in_=nw_bcast)

            cos_sb = consts.tile([n_win, rd // 2], F32)
            nc.sync.dma_start(out=cos_sb, in_=cos_h[:, :])
            sin_sb = consts.tile([n_win, rd // 2], F32)
            nc.sync.dma_start(out=sin_sb, in_=sin_h[:, :])

            # ---- weights + first x token-group, interleaved per chunk so
            # window 0's chunk-c matmuls unblock as soon as chunk c lands
            wcat_view = wcat_h[:, :].rearrange("(c p) h -> c p h", p=128)
            xT_view = xT_h[:, :].rearrange("(c p) t -> c p t", p=128)

            w_c = []
            xt = [[None] * NG for _ in range(DC)]
            for c in range(DC):
                t = wpool.tile([128, 2 * head], BF16, tag=f"w{c}")
                nc.gpsimd.dma_start(out=t, in_=wcat_view[c])
                w_c.append(t)
                t = xpool.tile([128, GT], BF16, tag=f"x{c}")
                nc.sync.dma_start(out=t, in_=xT_view[c][:, 0:GT])
                xt[c][0] = t
            for g in range(1, NG):
                for c in range(DC):
                    t = xpool.tile([128, GT], BF16, tag=f"x{c}")
                    nc.sync.dma_start(out=t, in_=xT_view[c][:, g * GT : (g + 1) * GT])
                    xt[c][g] = t

            # persistent PSUM collectors for pooled numerator / denominator
            den_ps = ps_v.tile([n_win, head], F32, tag="den")
            num_ps = ps_v.tile([n_win, head], F32, tag="num")

            pend = None  # deferred pooling matmuls (previous window)
            for w in range(n_win):
                g = w // WPG
                toff = (w % WPG) * RATIO

                kv_ps = ps_mm.tile([RATIO, head], F32, tag="kv")
                sc_ps = ps_mm.tile([RATIO, head], F32, tag="sc")

                for c in range(DC):
                    lhsT = xt[c][g][:, toff : toff + RATIO]
                    nc.tensor.matmul(
                        kv_ps, lhsT, w_c[c][:, 0:head],
                        start=(c == 0), stop=(c == DC - 1),
                    )
                    nc.tensor.matmul(
                        sc_ps, lhsT, w_c[c][:, head : 2 * head],
                        start=(c == 0), stop=False,
                    )
                # fold the ape bias into the sc accumulation
                nc.tensor.matmul(
                    sc_ps, ident, ape_sb[:, head : 2 * head], start=False, stop=True
                )

                # pooling matmuls for the PREVIOUS window go after this
                # window's projections so the PE never waits on exp/mul
                if pend is not None:
                    pend()

                e = em_p.tile([RATIO, head], BF16, tag="e")
                nc.scalar.activation(e, sc_ps, mybir.ActivationFunctionType.Exp)
                m = em_p.tile([RATIO, head], BF16, tag="m")
                nc.vector.tensor_mul(m, e, kv_ps)

                ind = big[:, n_win * w : n_win * w + n_win]

                def pend(w=w, e=e, m=m, ind=ind):
                    nc.tensor.matmul(
                        den_ps, ind, e,
                        start=(w == 0), stop=(w == n_win - 1),
                    )
                    nc.tensor.matmul(
                        num_ps, ind, m,
                        start=(w == 0), stop=(w == n_win - 1),
                    )

            pend()  # last window's pooling

            # ---- epilogue (batched over all n_win windows) ----
            dens = epi_p.tile([n_win, head], F32, tag="ep")
            nc.any.tensor_copy(dens, den_ps)
            rden = epi_p.tile([n_win, head], F32, tag="ep")
            nc.vector.reciprocal(rden, dens)
            comp = epi_p.tile([n_win, head], F32, tag="ep")
            nc.vector.tensor_mul(comp, num_ps, rden)

            # RMSNorm: rstd = 1/sqrt(mean(comp^2) + eps)
            sq = epi_p.tile([n_win, head], F32, tag="ep")
            ssq = sm_p.tile([n_win, 1], F32, tag="ssq")
            nc.scalar.activation(
                sq, comp, mybir.ActivationFunctionType.Square, accum_out=ssq
            )
            eps_sb = sm_p.tile([n_win, 1], F32, tag="eps")
            nc.vector.memset(eps_sb, EPS)
            srt = sm_p.tile([n_win, 1], F32, tag="srt")
            nc.scalar.activation(
                srt, ssq, mybir.ActivationFunctionType.Sqrt,
                scale=1.0 / head, bias=eps_sb,
            )
            rstd = sm_p.tile([n_win, 1], F32, tag="rstd")
            nc.vector.reciprocal(rstd, srt)

            compn = epi_p.tile([n_win, head], F32, tag="ep")
            nc.scalar.activation(
                compn, comp, mybir.ActivationFunctionType.Copy, scale=rstd
            )
            final = epi_p.tile([n_win, head], F32, tag="ep")
            nc.vector.tensor_mul(final, compn, nw_sb)

            # RoPE on last rd dims, pairs interleaved along free dim
            pr = final[:, head - rd : head].rearrange("p (k two) -> p k two", two=2)
            ev = pr[:, :, 0]
            od = pr[:, :, 1]
            t1 = sm_p.tile([n_win, rd // 2], F32, tag="t1")
            t2 = sm_p.tile([n_win, rd // 2], F32, tag="t2")
            t3 = sm_p.tile([n_win, rd // 2], F32, tag="t3")
            t4 = sm_p.tile([n_win, rd // 2], F32, tag="t4")
            nc.vector.tensor_mul(t1, ev, cos_sb)
            nc.vector.tensor_mul(t2, od, sin_sb)
            nc.vector.tensor_mul(t3, ev, sin_sb)
            nc.vector.tensor_mul(t4, od, cos_sb)
            nc.vector.tensor_sub(ev, t1, t2)
            nc.vector.tensor_add(od, t3, t4)

            nc.sync.dma_start(out=out_h[:, :], in_=final)

    nc.compile()
    return nc


def TileProgram(nc):
    return tile.TileContext(nc)


_cache = {}


def _get_program():
    if "nc" not in _cache:
        _cache["nc"] = build_program()
    return _cache["nc"]


def make_input_maps(x, wkv, wgate, ape, norm_weight, cos, sin):
    xf = np.asarray(x, np.float32).reshape(B * S, D).astype(BF16_NP)
    wcat = np.ascontiguousarray(
        np.concatenate(
            [np.asarray(wkv, np.float32), np.asarray(wgate, np.float32)], axis=1
        ).astype(BF16_NP)
    )
    apecat = np.zeros((RATIO, 2 * HEAD), np.float32)
    apecat[:, HEAD:] = np.asarray(ape, np.float32)
    apecat = np.ascontiguousarray(apecat.astype(BF16_NP))
    maps = []
    for i in range(N_CORES):
        g0 = i * N_WIN
        # position of window start within its sequence
        pos = ((np.arange(g0, g0 + N_WIN) % NC_PER_SEQ) * RATIO).astype(np.int64)
        maps.append(
            {
                "xT": np.ascontiguousarray(xf[i * TOK : (i + 1) * TOK].T),
                "wcat": wcat,
                "apecat": apecat,
                "nw": np.ascontiguousarray(norm_weight),
                "cosr": np.ascontiguousarray(cos[pos]),
                "sinr": np.ascontiguousarray(sin[pos]),
            }
        )
    return maps


def assemble_output(results, block_offsets):
    cache = np.zeros((NUM_BLOCKS, ENTRIES, HEAD), np.float32)
    bo = np.asarray(block_offsets)
    for i in range(N_CORES):
        comp = results[i]["out"]  # [N_WIN, HEAD]
        g0 = i * N_WIN
        b = g0 // NC_PER_SEQ
        c0 = g0 % NC_PER_SEQ
        cache[int(bo[b, 0]), c0 : c0 + N_WIN, :] = comp
    return cache


def _ensure_ntff_hook():
    """Register the axon NTFF profile hook if the image's antenv lacks it."""
    import types

    try:
        from antenv.axon_hooks import get_axon_ntff_profile_hook  # noqa: F401

        return
    except ImportError:
        pass
    import antenv
    from trn_agent_boot.trn_boot import _ntff_profile_via_ctypes

    hook = _ntff_profile_via_ctypes("/opt/axon/libaxon_pjrt.so")
    mod = types.ModuleType("antenv.axon_hooks")
    state = {"hook": hook}
    mod.get_axon_ntff_profile_hook = lambda: state["hook"]
    mod.set_axon_ntff_profile_hook = lambda h: state.__setitem__("hook", h)
    sys.modules["antenv.axon_hooks"] = mod
    antenv.axon_hooks = mod


def run_full(inputs, trace=False):
    # NOTE: walrus --enable-ldw-opt stays false (the default): the LDW
    # optimization pass rejects 16-bit InstLdweights.
    from concourse.bass_utils import run_bass_kernel_spmd

    if trace:
        _ensure_ntff_hook()
    nc = _get_program()
    maps = make_input_maps(
        inputs["x"], inputs["wkv"], inputs["wgate"], inputs["ape"],
        inputs["norm_weight"], inputs["cos"], inputs["sin"],
    )
    r = run_bass_kernel_spmd(nc, maps, list(range(N_CORES)), trace=trace)
    out = assemble_output(r.results, inputs["block_offsets"])
    return out, r


def kernel(**inputs):
    inputs = {k: np.asarray(v) for k, v in inputs.items()}
    out, _ = run_full(inputs, trace=False)
    return out


# revision 22
# speedup vs baseline: 1.6376x; 1.0260x over previous
"""Trainium2 Bass kernel for nn_Compressor (sparse_attention compressor).

Computes, for x [B=2, S=8192, D=4096]:
  kv = x @ wkv, sc = x @ wgate               (HEAD_DIM=512)
  w  = softmax(sc + ape, axis=window-of-128) (windows of RATIO=128 tokens)
  comp = sum(w * kv, axis=window)            -> [B, 64, 512]
  RMSNorm(comp) * norm_weight, RoPE on last 64 dims at window-start pos
  scatter into paged cache [4, 64, 512] via block_offsets

Sharding: 128 windows (B*NC) split across 8 cores, 16 windows each.
Pure data parallel - weights replicated, no collectives.

x is pre-transposed on the host to [D, tokens] per core, so the device
does no transposes: each 128-d chunk of xT is the matmul stationary
directly. All matmul operands are bf16 (host-cast). The ape bias is
folded into the sc accumulation as an extra identity-stationary matmul.
Per-window softmax pooling runs one window behind the projections so
the PE never waits on the exp/mul chain.
"""

import sys

sys.path.insert(0, "/opt/trn_rl_repo")

import numpy as np
import ml_dtypes
from contextlib import ExitStack

import concourse.bass as bass
import concourse.tile as tile
from concourse import bacc, mybir

# Problem constants (hardcoded per contract - kernel.py must be self-contained)
B = 2
S = 8192
D = 4096
HEAD = 512
RATIO = 128
RD = 64
EPS = 1e-6
NC_PER_SEQ = S // RATIO  # 64
BLOCK_SIZE = 8192
ENTRIES = 64
NUM_BLOCKS = 4
N_CORES = 8
N_WIN_TOTAL = B * NC_PER_SEQ  # 128
N_WIN = N_WIN_TOTAL // N_CORES  # 16 windows per core
TOK = N_WIN * RATIO  # 2048 tokens per core

F32 = mybir.dt.float32
BF16 = mybir.dt.bfloat16
BF16_NP = ml_dtypes.bfloat16


def build_program(n_win=N_WIN, d=D, head=HEAD, rd=RD, debug=False):
    """Build the per-core Bass program (SPMD: identical on all cores)."""
    assert d % 128 == 0 and head == 512
    DC = d // 128  # 32 contraction chunks
    tok = n_win * RATIO
    NG = 8  # token groups for x DMA granularity
    GT = tok // NG  # 256 tokens per group
    WPG = GT // RATIO  # windows per group

    nc = bacc.Bacc(None, target_bir_lowering=False, debug=debug)

    xT_h = nc.declare_dram_parameter("xT", [d, tok], BF16, isOutput=False)
    # wcat = [wkv | wgate] fused along head dim -> one 2KB-row DMA per chunk
    wcat_h = nc.declare_dram_parameter("wcat", [d, 2 * head], BF16, isOutput=False)
    ape_h = nc.declare_dram_parameter("ape", [RATIO, head], BF16, isOutput=False)
    nw_h = nc.declare_dram_parameter("nw", [head], F32, isOutput=False)
    cos_h = nc.declare_dram_parameter("cosr", [n_win, rd // 2], F32, isOutput=False)
    sin_h = nc.declare_dram_parameter("sinr", [n_win, rd // 2], F32, isOutput=False)
    out_h = nc.declare_dram_parameter("out", [n_win, head], F32, isOutput=True)

    with TileProgram(nc) as tc:
        with ExitStack() as ctx:
            consts = ctx.enter_context(tc.tile_pool(name="consts", bufs=1))
            xpool = ctx.enter_context(tc.tile_pool(name="xpool", bufs=6))
            wpool = ctx.enter_context(tc.tile_pool(name="wpool", bufs=1))
            em_p = ctx.enter_context(tc.tile_pool(name="em", bufs=2))
            epi_p = ctx.enter_context(tc.tile_pool(name="epi", bufs=4))
            sm_p = ctx.enter_context(tc.tile_pool(name="sm", bufs=2))
            ps_mm = ctx.enter_context(tc.tile_pool(name="ps_mm", bufs=2, space="PSUM"))
            ps_v = ctx.enter_context(tc.tile_pool(name="ps_v", bufs=1, space="PSUM"))

            # ---- constants (scalar DMA queue: sync/gpsimd carry x/w) ----
            # striped indicator: big[:, (n_win+1)*w] column of ones; slice
            # big[:, n_win*w : n_win*w+n_win] is E_w with ones in column w.
            big0 = consts.tile([128, n_win * n_win], F32)
            nc.vector.memset(big0, 0.0)
            for w in range(n_win):
                nc.vector.memset(big0[:, (n_win + 1) * w : (n_win + 1) * w + 1], 1.0)
            big = consts.tile([128, n_win * n_win], BF16)
            nc.vector.tensor_copy(big, big0)

            ape_sb = consts.tile([RATIO, head], BF16)
            nc.scalar.dma_start(out=ape_sb, in_=ape_h[:, :])

            # norm_weight broadcast across the n_win partitions
            nw_sb = consts.tile([n_win, head], F32)
            nw_ap = nw_h[:]
            nw_bcast = bass.AP(
                tensor=nw_ap.tensor, offset=nw_ap.offset,
                ap=[[0, n_win]] + list(nw_ap.ap),
            )
            nc.scalar.dma_start(out=nw_sb, in_=nw_bcast)

            cos_sb = consts.tile([n_win, rd // 2], F32)
            nc.scalar.dma_start(out=cos_sb, in_=cos_h[:, :])
            sin_sb = consts.tile([n_win, rd // 2], F32)
            nc.scalar.dma_start(out=sin_sb, in_=sin_h[:, :])

            eps_sb = consts.tile([n_win, 1], F32)
            nc.vector.memset(eps_sb, EPS)

            # ---- weights + x, spread over the sync and gpsimd DMA queues.
            # sync: x group 0 (chunk-ascending, window 0 consumes in order),
            # then even groups; gpsimd: w chunks, then odd groups. The exp/
            # mul engines (scalar/vector) carry no x DMAs so ring-buffer
            # waits can never stall the softmax chain.
            wcat_view = wcat_h[:, :].rearrange("(c p) h -> c p h", p=128)
            xT_view = xT_h[:, :].rearrange("(c p) t -> c p t", p=128)

            w_c = []
            xt = [[None] * NG for _ in range(DC)]

            def x_dma(eng, c, g):
                t = xpool.tile([128, GT], BF16, tag=f"x{c}")
                eng.dma_start(out=t, in_=xT_view[c][:, g * GT : (g + 1) * GT])
                xt[c][g] = t

            for c in range(DC):
                t = wpool.tile([128, 2 * head], BF16, tag=f"w{c}")
                nc.gpsimd.dma_start(out=t, in_=wcat_view[c])
                w_c.append(t)
                x_dma(nc.sync, c, 0)
            # remaining groups: odd -> gpsimd (behind w), even -> sync
            for g in range(1, NG):
                eng = nc.gpsimd if g % 2 == 1 else nc.sync
                for c in range(DC):
                    x_dma(eng, c, g)

            # persistent PSUM collectors for pooled numerator / denominator
            den_ps = ps_v.tile([n_win, head], F32, tag="den")
            num_ps = ps_v.tile([n_win, head], F32, tag="num")

            pend = None  # deferred pooling matmuls (previous window)
            for w in range(n_win):
                g = w // WPG
                toff = (w % WPG) * RATIO

                kv_ps = ps_mm.tile([RATIO, head], F32, tag="kv")
                sc_ps = ps_mm.tile([RATIO, head], F32, tag="sc")

                for c in range(DC):
                    lhsT = xt[c][g][:, toff : toff + RATIO]
                    nc.tensor.matmul(
                        kv_ps, lhsT, w_c[c][:, 0:head],
                        start=(c == 0), stop=(c == DC - 1),
                    )
                    nc.tensor.matmul(
                        sc_ps, lhsT, w_c[c][:, head : 2 * head],
                        start=(c == 0), stop=(c == DC - 1),
                    )

                # pooling matmuls for the PREVIOUS window go after this
                # window's projections so the PE never waits on exp/mul
                if pend is not None:
                    pend()

                se = em_p.tile([RATIO, head], F32, tag="se")
                nc.vector.tensor_add(se, sc_ps, ape_sb)
                e = em_p.tile([RATIO, head], BF16, tag="e")
                nc.scalar.activation(e, se, mybir.ActivationFunctionType.Exp)
                m = em_p.tile([RATIO, head], BF16, tag="m")
                nc.vector.tensor_mul(m, e, kv_ps)

                ind = big[:, n_win * w : n_win * w + n_win]

                def pend(w=w, e=e, m=m, ind=ind):
                    nc.tensor.matmul(
                        den_ps, ind, e,
                        start=(w == 0), stop=(w == n_win - 1),
                    )
                    nc.tensor.matmul(
                        num_ps, ind, m,
                        start=(w == 0), stop=(w == n_win - 1),
                    )

            pend()  # last window's pooling

            # ---- epilogue (batched over all n_win windows) ----
            rden = epi_p.tile([n_win, head], F32, tag="ep")
            nc.vector.reciprocal(rden, den_ps)
            comp = epi_p.tile([n_win, head], F32, tag="ep")
            nc.vector.tensor_mul(comp, num_ps, rden)

            # RMSNorm: rstd = 1/sqrt(mean(comp^2) + eps)
            sq = epi_p.tile([n_win, head], F32, tag="ep")
            ssq = sm_p.tile([n_win, 1], F32, tag="ssq")
            nc.scalar.activation(
                sq, comp, mybir.ActivationFunctionType.Square, accum_out=ssq
            )
            srt = sm_p.tile([n_win, 1], F32, tag="srt")
            nc.scalar.activation(
                srt, ssq, mybir.ActivationFunctionType.Sqrt,
                scale=1.0 / head, bias=eps_sb,
            )
            rstd = sm_p.tile([n_win, 1], F32, tag="rstd")
            nc.vector.reciprocal(rstd, srt)
            compn = epi_p.tile([n_win, head], F32, tag="ep")
            nc.vector.tensor_scalar_mul(compn, comp, rstd)
            final = epi_p.tile([n_win, head], F32, tag="ep")
            nc.vector.tensor_mul(final, compn, nw_sb)

            # RoPE on last rd dims, pairs interleaved along free dim
            pr = final[:, head - rd : head].rearrange("p (k two) -> p k two", two=2)
            ev = pr[:, :, 0]
            od = pr[:, :, 1]
            t1 = sm_p.tile([n_win, rd // 2], F32, tag="t1")
            t2 = sm_p.tile([n_win, rd // 2], F32, tag="t2")
            t3 = sm_p.tile([n_win, rd // 2], F32, tag="t3")
            t4 = sm_p.tile([n_win, rd // 2], F32, tag="t4")
            nc.vector.tensor_mul(t1, ev, cos_sb)
            nc.vector.tensor_mul(t2, od, sin_sb)
            nc.vector.tensor_mul(t3, ev, sin_sb)
            nc.vector.tensor_mul(t4, od, cos_sb)
            nc.vector.tensor_sub(ev, t1, t2)
            nc.vector.tensor_add(od, t3, t4)

            nc.sync.dma_start(out=out_h[:, :], in_=final)

    nc.compile()
    return nc


def TileProgram(nc):
    return tile.TileContext(nc)


_cache = {}


def _get_program():
    if "nc" not in _cache:
        _cache["nc"] = build_program()
    return _cache["nc"]


def make_input_maps(x, wkv, wgate, ape, norm_weight, cos, sin):
    xf = np.asarray(x, np.float32).reshape(B * S, D).astype(BF16_NP)
    wcat = np.ascontiguousarray(
        np.concatenate(
            [np.asarray(wkv, np.float32), np.asarray(wgate, np.float32)], axis=1
        ).astype(BF16_NP)
    )
    ape_b = np.ascontiguousarray(np.asarray(ape, np.float32).astype(BF16_NP))
    maps = []
    for i in range(N_CORES):
        g0 = i * N_WIN
        # position of window start within its sequence
        pos = ((np.arange(g0, g0 + N_WIN) % NC_PER_SEQ) * RATIO).astype(np.int64)
        maps.append(
            {
                "xT": np.ascontiguousarray(xf[i * TOK : (i + 1) * TOK].T),
                "wcat": wcat,
                "ape": ape_b,
                "nw": np.ascontiguousarray(norm_weight),
                "cosr": np.ascontiguousarray(cos[pos]),
                "sinr": np.ascontiguousarray(sin[pos]),
            }
        )
    return maps


def assemble_output(results, block_offsets):
    cache = np.zeros((NUM_BLOCKS, ENTRIES, HEAD), np.float32)
    bo = np.asarray(block_offsets)
    for i in range(N_CORES):
        comp = results[i]["out"]  # [N_WIN, HEAD]
        g0 = i * N_WIN
        b = g0 // NC_PER_SEQ
        c0 = g0 % NC_PER_SEQ
        cache[int(bo[b, 0]), c0 : c0 + N_WIN, :] = comp
    return cache


def _ensure_ntff_hook():
    """Register the axon NTFF profile hook if the image's antenv lacks it."""
    import types

    try:
        from antenv.axon_hooks import get_axon_ntff_profile_hook  # noqa: F401

        return
    except ImportError:
        pass
    import antenv
    from trn_agent_boot.trn_boot import _ntff_profile_via_ctypes

    hook = _ntff_profile_via_ctypes("/opt/axon/libaxon_pjrt.so")
    mod = types.ModuleType("antenv.axon_hooks")
    state = {"hook": hook}
    mod.get_axon_ntff_profile_hook = lambda: state["hook"]
    mod.set_axon_ntff_profile_hook = lambda h: state.__setitem__("hook", h)
    sys.modules["antenv.axon_hooks"] = mod
    antenv.axon_hooks = mod


def run_full(inputs, trace=False):
    # NOTE: walrus --enable-ldw-opt stays false (the default): the LDW
    # optimization pass rejects 16-bit InstLdweights.
    from concourse.bass_utils import run_bass_kernel_spmd

    if trace:
        _ensure_ntff_hook()
    nc = _get_program()
    maps = make_input_maps(
        inputs["x"], inputs["wkv"], inputs["wgate"], inputs["ape"],
        inputs["norm_weight"], inputs["cos"], inputs["sin"],
    )
    r = run_bass_kernel_spmd(nc, maps, list(range(N_CORES)), trace=trace)
    out = assemble_output(r.results, inputs["block_offsets"])
    return out, r


def kernel(**inputs):
    inputs = {k: np.asarray(v) for k, v in inputs.items()}
    out, _ = run_full(inputs, trace=False)
    return out


# revision 30
# speedup vs baseline: 1.6541x; 1.0101x over previous
"""Trainium2 Bass kernel for nn_Compressor (sparse_attention compressor).

Computes, for x [B=2, S=8192, D=4096]:
  kv = x @ wkv, sc = x @ wgate               (HEAD_DIM=512)
  w  = softmax(sc + ape, axis=window-of-128) (windows of RATIO=128 tokens)
  comp = sum(w * kv, axis=window)            -> [B, 64, 512]
  RMSNorm(comp) * norm_weight, RoPE on last 64 dims at window-start pos
  scatter into paged cache [4, 64, 512] via block_offsets

Sharding: 128 windows (B*NC) split across 8 cores, 16 windows each.
Pure data parallel - weights replicated, no collectives.

x is pre-transposed on the host to [D, tokens] per core, so the device
does no transposes: each 128-d chunk of xT is the matmul stationary
directly. All matmul operands are bf16 (host-cast). The ape bias is
folded into the sc accumulation as an extra identity-stationary matmul.
Per-window softmax pooling runs one window behind the projections so
the PE never waits on the exp/mul chain.
"""

import sys

sys.path.insert(0, "/opt/trn_rl_repo")

import numpy as np
import ml_dtypes
from contextlib import ExitStack

import concourse.bass as bass
import concourse.tile as tile
from concourse import bacc, mybir

# Problem constants (hardcoded per contract - kernel.py must be self-contained)
B = 2
S = 8192
D = 4096
HEAD = 512
RATIO = 128
RD = 64
EPS = 1e-6
NC_PER_SEQ = S // RATIO  # 64
BLOCK_SIZE = 8192
ENTRIES = 64
NUM_BLOCKS = 4
N_CORES = 8
N_WIN_TOTAL = B * NC_PER_SEQ  # 128
N_WIN = N_WIN_TOTAL // N_CORES  # 16 windows per core
TOK = N_WIN * RATIO  # 2048 tokens per core

F32 = mybir.dt.float32
BF16 = mybir.dt.bfloat16
BF16_NP = ml_dtypes.bfloat16


def build_program(n_win=N_WIN, d=D, head=HEAD, rd=RD, debug=False):
    """Build the per-core Bass program (SPMD: identical on all cores)."""
    assert d % 128 == 0 and head == 512
    DC = d // 128  # 32 contraction chunks
    tok = n_win * RATIO
    NG = 8  # token groups for x DMA granularity
    GT = tok // NG  # 256 tokens per group
    WPG = GT // RATIO  # windows per group

    nc = bacc.Bacc(None, target_bir_lowering=False, debug=debug)

    xT_h = nc.declare_dram_parameter("xT", [d, tok], BF16, isOutput=False)
    # wcat = [wkv | wgate] fused along head dim -> one 2KB-row DMA per chunk
    wcat_h = nc.declare_dram_parameter("wcat", [d, 2 * head], BF16, isOutput=False)
    ape_h = nc.declare_dram_parameter("ape", [RATIO, head], BF16, isOutput=False)
    nw_h = nc.declare_dram_parameter("nw", [head], F32, isOutput=False)
    cos_h = nc.declare_dram_parameter("cosr", [n_win, rd // 2], F32, isOutput=False)
    sin_h = nc.declare_dram_parameter("sinr", [n_win, rd // 2], F32, isOutput=False)
    out_h = nc.declare_dram_parameter("out", [n_win, head], F32, isOutput=True)

    with TileProgram(nc) as tc:
        with ExitStack() as ctx:
            consts = ctx.enter_context(tc.tile_pool(name="consts", bufs=1))
            xpool = ctx.enter_context(tc.tile_pool(name="xpool", bufs=6))
            wpool = ctx.enter_context(tc.tile_pool(name="wpool", bufs=1))
            em_p = ctx.enter_context(tc.tile_pool(name="em", bufs=2))
            epi_p = ctx.enter_context(tc.tile_pool(name="epi", bufs=1))
            sm_p = ctx.enter_context(tc.tile_pool(name="sm", bufs=2))
            ps_mm = ctx.enter_context(tc.tile_pool(name="ps_mm", bufs=2, space="PSUM"))
            ps_v = ctx.enter_context(tc.tile_pool(name="ps_v", bufs=1, space="PSUM"))

            # ---- constants (scalar DMA queue: sync/gpsimd carry x/w) ----
            # striped indicator: big[:, (n_win+1)*w] column of ones; slice
            # big[:, n_win*w : n_win*w+n_win] is E_w with ones in column w.
            big0 = consts.tile([128, n_win * n_win], F32)
            nc.vector.memset(big0, 0.0)
            for w in range(n_win):
                nc.vector.memset(big0[:, (n_win + 1) * w : (n_win + 1) * w + 1], 1.0)
            big = consts.tile([128, n_win * n_win], BF16)
            nc.vector.tensor_copy(big, big0)

            ape_sb = consts.tile([RATIO, head], BF16)
            nc.scalar.dma_start(out=ape_sb, in_=ape_h[:, :])

            # norm_weight broadcast across the n_win partitions
            nw_sb = consts.tile([n_win, head], F32)
            nw_ap = nw_h[:]
            nw_bcast = bass.AP(
                tensor=nw_ap.tensor, offset=nw_ap.offset,
                ap=[[0, n_win]] + list(nw_ap.ap),
            )
            nc.scalar.dma_start(out=nw_sb, in_=nw_bcast)

            HF = n_win // 2
            cosA_sb = consts.tile([HF, rd // 2], F32)
            nc.scalar.dma_start(out=cosA_sb, in_=cos_h[0:HF, :])
            cosB_sb = consts.tile([HF, rd // 2], F32)
            nc.scalar.dma_start(out=cosB_sb, in_=cos_h[HF : 2 * HF, :])
            sinA_sb = consts.tile([HF, rd // 2], F32)
            nc.scalar.dma_start(out=sinA_sb, in_=sin_h[0:HF, :])
            sinB_sb = consts.tile([HF, rd // 2], F32)
            nc.scalar.dma_start(out=sinB_sb, in_=sin_h[HF : 2 * HF, :])

            eps_sb = consts.tile([n_win, 1], F32)
            nc.vector.memset(eps_sb, EPS)

            # warm both activation tables now so no ACT_TABLE_LOAD lands on
            # the end-of-kernel critical path (Sqrt sits in a second table)
            scr1 = sm_p.tile([n_win, 1], F32, tag="scr1")
            nc.scalar.activation(scr1, eps_sb, mybir.ActivationFunctionType.Sqrt)
            scr2 = sm_p.tile([n_win, 1], F32, tag="scr2")
            nc.scalar.activation(scr2, eps_sb, mybir.ActivationFunctionType.Exp)

            # ---- weights + x, spread over the sync and gpsimd DMA queues.
            # sync: x group 0 (chunk-ascending, window 0 consumes in order),
            # then even groups; gpsimd: w chunks, then odd groups. The exp/
            # mul engines (scalar/vector) carry no x DMAs so ring-buffer
            # waits can never stall the softmax chain.
            wcat_view = wcat_h[:, :].rearrange("(c p) h -> c p h", p=128)
            xT_view = xT_h[:, :].rearrange("(c p) t -> c p t", p=128)

            w_c = []
            xt = [[None] * NG for _ in range(DC)]

            def x_dma(eng, c, g):
                t = xpool.tile([128, GT], BF16, tag=f"x{c}")
                eng.dma_start(out=t, in_=xT_view[c][:, g * GT : (g + 1) * GT])
                xt[c][g] = t

            for c in range(DC):
                t = wpool.tile([128, 2 * head], BF16, tag=f"w{c}")
                nc.gpsimd.dma_start(out=t, in_=wcat_view[c])
                w_c.append(t)
                x_dma(nc.sync, c, 0)
            # remaining groups: odd -> gpsimd (behind w), even -> sync
            for g in range(1, NG):
                eng = nc.gpsimd if g % 2 == 1 else nc.sync
                for c in range(DC):
                    x_dma(eng, c, g)

            # persistent PSUM collectors for pooled numerator / denominator,
            # split into window halves so half A's epilogue overlaps the
            # second half's projections
            H = n_win // 2
            den_h = [
                ps_v.tile([H, head], F32, tag="denA", name="denA"),
                ps_v.tile([H, head], F32, tag="denB", name="denB"),
            ]
            num_h = [
                ps_v.tile([H, head], F32, tag="numA", name="numA"),
                ps_v.tile([H, head], F32, tag="numB", name="numB"),
            ]

            finalA = consts.tile([H, head], F32, name="finalA")
            finalB = consts.tile([H, head], F32, name="finalB")

            def epi_half(tag, dh, nh, fin, c_sb, s_sb):
                """num/den -> RMSNorm -> RoPE for window rows r0..r0+H-1."""
                rden = epi_p.tile([H, head], F32, tag=f"rden{tag}")
                nc.vector.reciprocal(rden, dh)
                comp = epi_p.tile([H, head], F32, tag=f"comp{tag}")
                nc.vector.tensor_mul(comp, nh, rden)
                sq = epi_p.tile([H, head], F32, tag=f"sq{tag}")
                ssq = sm_p.tile([H, 1], F32, tag=f"ssq{tag}")
                nc.scalar.activation(
                    sq, comp, mybir.ActivationFunctionType.Square, accum_out=ssq
                )
                srt = sm_p.tile([H, 1], F32, tag=f"srt{tag}")
                nc.scalar.activation(
                    srt, ssq, mybir.ActivationFunctionType.Sqrt,
                    scale=1.0 / head, bias=eps_sb[0:H, :],
                )
                rstd = sm_p.tile([H, 1], F32, tag=f"rstd{tag}")
                nc.vector.reciprocal(rstd, srt)
                compn = epi_p.tile([H, head], F32, tag=f"compn{tag}")
                nc.vector.tensor_scalar_mul(compn, comp, rstd)
                nc.vector.tensor_mul(fin, compn, nw_sb[0:H, :])

                # RoPE on last rd dims, pairs interleaved along free dim
                pr = fin[:, head - rd : head].rearrange("p (k two) -> p k two", two=2)
                ev = pr[:, :, 0]
                od = pr[:, :, 1]
                t1 = sm_p.tile([H, rd // 2], F32, tag=f"t1{tag}")
                t2 = sm_p.tile([H, rd // 2], F32, tag=f"t2{tag}")
                t3 = sm_p.tile([H, rd // 2], F32, tag=f"t3{tag}")
                t4 = sm_p.tile([H, rd // 2], F32, tag=f"t4{tag}")
                nc.vector.tensor_mul(t1, ev, c_sb)
                nc.vector.tensor_mul(t2, od, s_sb)
                nc.vector.tensor_mul(t3, ev, s_sb)
                nc.vector.tensor_mul(t4, od, c_sb)
                nc.vector.tensor_sub(ev, t1, t2)
                nc.vector.tensor_add(od, t3, t4)

            pend = None  # deferred pooling matmuls (previous window)
            for w in range(n_win):
                g = w // WPG
                toff = (w % WPG) * RATIO

                kv_ps = ps_mm.tile([RATIO, head], F32, tag="kv")
                sc_ps = ps_mm.tile([RATIO, head], F32, tag="sc")

                for c in range(DC):
                    lhsT = xt[c][g][:, toff : toff + RATIO]
                    nc.tensor.matmul(
                        kv_ps, lhsT, w_c[c][:, 0:head],
                        start=(c == 0), stop=(c == DC - 1),
                    )
                    nc.tensor.matmul(
                        sc_ps, lhsT, w_c[c][:, head : 2 * head],
                        start=(c == 0), stop=(c == DC - 1),
                    )

                # pooling matmuls for the PREVIOUS window go after this
                # window's projections so the PE never waits on exp/mul
                if pend is not None:
                    pend()
                if w == H:
                    # half A's pools are complete: overlap its epilogue with
                    # the second half's projections
                    epi_half("A", den_h[0], num_h[0], finalA, cosA_sb, sinA_sb)

                se = em_p.tile([RATIO, head], F32, tag="se")
                nc.vector.tensor_add(se, sc_ps, ape_sb)
                e = em_p.tile([RATIO, head], BF16, tag="e")
                nc.scalar.activation(e, se, mybir.ActivationFunctionType.Exp)
                m = em_p.tile([RATIO, head], BF16, tag="m")
                nc.vector.tensor_mul(m, e, kv_ps)

                half = 0 if w < H else 1
                ind = big[:, n_win * w + H * half : n_win * w + H * half + H]

                def pend(w=w, e=e, m=m, ind=ind, half=half):
                    nc.tensor.matmul(
                        den_h[half], ind, e,
                        start=(w % H == 0), stop=(w % H == H - 1),
                    )
                    nc.tensor.matmul(
                        num_h[half], ind, m,
                        start=(w % H == 0), stop=(w % H == H - 1),
                    )

            pend()  # last window's pooling
            epi_half("B", den_h[1], num_h[1], finalB, cosB_sb, sinB_sb)

            nc.sync.dma_start(out=out_h[0:H, :], in_=finalA)
            nc.sync.dma_start(out=out_h[H : 2 * H, :], in_=finalB)

    nc.compile()
    return nc


def TileProgram(nc):
    return tile.TileContext(nc)


_cache = {}


def _get_program():
    if "nc" not in _cache:
        _cache["nc"] = build_program()
    return _cache["nc"]


def make_input_maps(x, wkv, wgate, ape, norm_weight, cos, sin):
    xf = np.asarray(x, np.float32).reshape(B * S, D).astype(BF16_NP)
    wcat = np.ascontiguousarray(
        np.concatenate(
            [np.asarray(wkv, np.float32), np.asarray(wgate, np.float32)], axis=1
        ).astype(BF16_NP)
    )
    ape_b = np.ascontiguousarray(np.asarray(ape, np.float32).astype(BF16_NP))
    maps = []
    for i in range(N_CORES):
        g0 = i * N_WIN
        # position of window start within its sequence
        pos = ((np.arange(g0, g0 + N_WIN) % NC_PER_SEQ) * RATIO).astype(np.int64)
        maps.append(
            {
                "xT": np.ascontiguousarray(xf[i * TOK : (i + 1) * TOK].T),
                "wcat": wcat,
                "ape": ape_b,
                "nw": np.ascontiguousarray(norm_weight),
                "cosr": np.ascontiguousarray(cos[pos]),
                "sinr": np.ascontiguousarray(sin[pos]),
            }
        )
    return maps


def assemble_output(results, block_offsets):
    cache = np.zeros((NUM_BLOCKS, ENTRIES, HEAD), np.float32)
    bo = np.asarray(block_offsets)
    for i in range(N_CORES):
        comp = results[i]["out"]  # [N_WIN, HEAD]
        g0 = i * N_WIN
        b = g0 // NC_PER_SEQ
        c0 = g0 % NC_PER_SEQ
        cache[int(bo[b, 0]), c0 : c0 + N_WIN, :] = comp
    return cache


def _ensure_ntff_hook():
    """Register the axon NTFF profile hook if the image's antenv lacks it."""
    import types

    try:
        from antenv.axon_hooks import get_axon_ntff_profile_hook  # noqa: F401

        return
    except ImportError:
        pass
    import antenv
    from trn_agent_boot.trn_boot import _ntff_profile_via_ctypes

    hook = _ntff_profile_via_ctypes("/opt/axon/libaxon_pjrt.so")
    mod = types.ModuleType("antenv.axon_hooks")
    state = {"hook": hook}
    mod.get_axon_ntff_profile_hook = lambda: state["hook"]
    mod.set_axon_ntff_profile_hook = lambda h: state.__setitem__("hook", h)
    sys.modules["antenv.axon_hooks"] = mod
    antenv.axon_hooks = mod


def run_full(inputs, trace=False):
    # NOTE: walrus --enable-ldw-opt stays false (the default): the LDW
    # optimization pass rejects 16-bit InstLdweights.
    from concourse.bass_utils import run_bass_kernel_spmd

    if trace:
        _ensure_ntff_hook()
    nc = _get_program()
    maps = make_input_maps(
        inputs["x"], inputs["wkv"], inputs["wgate"], inputs["ape"],
        inputs["norm_weight"], inputs["cos"], inputs["sin"],
    )
    r = run_bass_kernel_spmd(nc, maps, list(range(N_CORES)), trace=trace)
    out = assemble_output(r.results, inputs["block_offsets"])
    return out, r


def kernel(**inputs):
    inputs = {k: np.asarray(v) for k, v in inputs.items()}
    out, _ = run_full(inputs, trace=False)
    return out
